# revision 43
# baseline (speedup 1.0000x reference)
"""Single-head attention block (Q/K/V/O projections + softmax attention) on
8 Trainium2 NeuronCores.

Problem: x [16, 2048, 512] fp32; four 512x512 projections (torch convention
y = x @ W.T + b); scores = Q @ K.T / sqrt(512); softmax over keys;
out = attn @ V; y = out @ Wo.T + bo.

Sharding: pure data-parallel over batch -- each of the 8 cores computes 2 of
the 16 batches end-to-end. No collectives.

Algebraic restructuring (softmax is invariant to adding any function of the
query row, so those terms are dropped):
  scores ~ x A x^T + w[k]   with A = Wq^T Wk / sqrt(D), w = x (Wk^T bq)/sqrt(D)
  y = attn x B / rowsum + c with B = Wv^T Wo^T, c = bv Wo^T + bo
This removes the Q, K and V projections entirely.

Numerics: the whole attention inner loop runs fp8(e4m3) DoubleRow (2x PE
rate), with the O-projection epilogue in bf16 (rel-err gate is 2e-2; this
scheme lands ~1.69e-2 on HW):
  * HT[d',q] = fp8(16*SCALE*((x A)[q,d'] + v[d'])) via BF16 matmuls
    (A16 = bf16(16*SCALE*Wq^T Wk) x xTbf): same PE cycles as the fp8-DR
    Ah+Al error-feedback pair it replaced, but the A-quantization and
    query-side x errors drop to bf16 level; only HT's output is fp8. The
    w[k] exp bias is folded into HT via the eviction ACT's per-partition
    bias operand -- pss[k,q] = sum_d' x8[k,d'] HT'[d',q] reproduces
    16*(scores + w[k]) because the v term rides the same x8[k,:]
    contraction. This deletes the whole w-column pipeline (psw matmuls,
    w_row, 16 row->col transposes per batch) and the exp bias operand, at
    zero accuracy cost (v is absorbed into HT's existing fp8 rounding).
  * exp -> f32 on ACT (its native output path), then one DVE RNE cast to
    fp8 (A8_DIRECT fp8-out of ACT rounds coarser on HW: +~0.15e-2 rel err;
    a bf16 intermediate double-rounds: +~0.1e-2).
  * ZT accumulation is fp8 DR over the SAME a8 tiles the rowsum uses
    (numerator and denominator see identical quantized weights, so softmax
    normalization stays self-consistent): po[dt] += x8N-pairs.T @ a8-pairs.
  * rowsum: ones-weight fp8 DR over a8, all 8 pairs.
  * epilogue (y = oc @ B / rs + c) stays bf16: fp8 there costs ~9e-3 rel err.
x is resident in two fp8 layouts: xT8 [128, ND*S] d-tile-major (PE-transposed)
and x8N [128, NS*D] s-tile-major (direct DVE cast; po DR weights).

The four weight matrices AND x are pre-cast to bf16 on the HOST (for the
weights this is the identical RNE rounding the on-device DVE cast applied;
for x it costs ~+0.1e-2 rel err via fp8(bf16(x)) double rounding on the
x8N path): halves the weight + x DMA bytes, removes the staging casts from
the A = Wq^T Wk critical chain, and lets the PE transposes read the staged
x tile directly (the DVE staging cast they replaced was the batch-boundary
stall mechanism). All setup matmuls run bf16 (4x the f32r rate; A/B are
re-quantized to fp8/bf16 downstream so the bf16 input rounding is
absorbed).

Schedule: per kt-PAIR the PE stream is one contiguous fp8-DR block: next
pair's scores + prev pair's rowsum + prev pair's ZT (+ JIT HT at p==3). The
only bf16 islands are the deferred per-chunk epilogue (flushed at p==1) and
the prefetch x-transposes, batched next to the islands where possible.
Chunk boundaries are software-pipelined two ways: the NEXT chunk's (or
next batch's) first scores pair is emitted at the current chunk's tail, and
the current chunk's LAST-pair rowsum/ZT + PSUM evictions defer into the
next chunk's p==0 block (~4us after that pair's exp was issued), so no
chunk boundary waits on the exp->cast chain. The next batch's
HT(0) is emitted from inside the previous batch's last chunk. The final
chunk's y writeback is split per-q-tile so it streams out as the epilogue
computes. fp8<->bf16 weight-dtype switches cost ~100-200ns each on HW.
  * Every [1,128]->[128,1] row transpose runs bf16: a true-FP32 transpose is
    a multi-pass FP32_HI weight load and WEDGES the PE (device-unrecoverable
    hang) when interleaved with fp8 weight loads; f32r can't do 1-col
    transposes at all.
  * DMAs are batched 4-tiles-per-descriptor; y writeback issues from the
    idle gpsimd queue. A bf16 warmup burst at kernel start flips the PE HAM
    clock-gate to 8/8 (2.4 GHz) while the first DMAs are in flight.
"""

import os
from contextlib import ExitStack

import numpy as np
import ml_dtypes

import concourse.bass as bass
import concourse.tile as tile
from concourse import bacc, mybir
from concourse.bass_utils import run_bass_kernel_spmd
from concourse.masks import make_identity

N_CORES = 8
B, S, D = 16, 2048, 512
BPC = B // N_CORES  # batches per core
P = 128
ND = D // P         # 4   tiles over d/e/f dims
NS = S // P         # 16  tiles over s (= q = k) dim
QC = 512            # s/q-chunk width (PSUM bank)
NQC = S // QC       # 4
TPC = QC // P       # 4   128-tiles per chunk
NP = NS // 2        # 8   kt-pairs per chunk
SCALE = float(1.0 / np.sqrt(D))
HT_SC = 16.0           # HT fp8 tiles hold 16*(x A + v); exp applies 1/16

F32 = mybir.dt.float32
F32R = mybir.dt.float32r
F8 = mybir.dt.float8e4
BF16 = mybir.dt.bfloat16
AFT = mybir.ActivationFunctionType
ALU = mybir.AluOpType
DR = mybir.MatmulPerfMode.DoubleRow
A8_DIRECT = False  # fp8 straight out of ACT exp: +~0.15e-2 rel err on HW
                   # (coarser rounding than the DVE bf16->fp8 cast path)


def _emit(tc, x_ap, w_aps, b_aps, y_ap):
    nc = tc.nc
    ctx = ExitStack()
    with ctx:
        # ---- pools ----
        consts = ctx.enter_context(tc.tile_pool(name="consts", bufs=1))
        stage = ctx.enter_context(tc.tile_pool(name="stage", bufs=6))
        wset = ctx.enter_context(tc.tile_pool(name="wset", bufs=3))
        ab_pool = ctx.enter_context(tc.tile_pool(name="ab", bufs=1))
        xt_pool = ctx.enter_context(tc.tile_pool(name="xt", bufs=2))
        x8n_pool = ctx.enter_context(tc.tile_pool(name="x8n", bufs=2))
        xtb_pool = ctx.enter_context(tc.tile_pool(name="xtb", bufs=4))
        ht_pool = ctx.enter_context(tc.tile_pool(name="ht", bufs=2))
        oc_pool = ctx.enter_context(tc.tile_pool(name="oc", bufs=3))
        at_pool = ctx.enter_context(tc.tile_pool(name="at", bufs=5))
        at8_pool = ctx.enter_context(tc.tile_pool(name="at8", bufs=4))
        y_pool = ctx.enter_context(tc.tile_pool(name="y", bufs=2))
        rs_pool = ctx.enter_context(tc.tile_pool(name="rs", bufs=2))
        ppt = ctx.enter_context(tc.tile_pool(name="ppt", bufs=3, space="PSUM"))
        ppo = ctx.enter_context(tc.tile_pool(name="ppo", bufs=4, space="PSUM"))
        ppr = ctx.enter_context(tc.tile_pool(name="ppr", bufs=1, space="PSUM"))

        def pt_tile():
            return ppt.tile([P, QC], F32, tag="ppt", name="pt")

        # ---- constants ----
        ones_bf = consts.tile([P, P], BF16, tag="ones_bf")
        nc.vector.memset(ones_bf[:], 1.0)

        def filler(n=1):
            # bf16 no-op matmuls that keep the PE HAM activity window busy
            # through DMA-bound stretches so the clock gate stays at 2.4 GHz
            for _ in range(n):
                ps = pt_tile()
                nc.tensor.matmul(
                    ps[:, 0:P], ones_bf[:], ones_bf[:], start=True, stop=True
                )

        # Dense matmul burst: ~2us of sustained PE activity flips the PE HAM
        # clock-gate to 8/8 (2.4 GHz); the rest covers the startup HBM burst
        # (all 8 cores DMA weights + first x chunks simultaneously).
        filler(44)
        ident = consts.tile([P, P], F32, tag="ident")
        make_identity(nc, ident[:])
        ident_bf = consts.tile([P, P], BF16, tag="ident_bf")
        nc.vector.tensor_copy(ident_bf[:], ident[:])
        ones8 = consts.tile([P, 2 * P], F8, tag="ones8")
        nc.vector.memset(ones8[:], 1.0)
        ones_row_f = consts.tile([1, P], F32, tag="ones_row_f")
        nc.vector.memset(ones_row_f[:], 1.0)
        ones_row_r = consts.tile([1, P], F32R, tag="ones_row_r")
        nc.vector.tensor_copy(ones_row_r[:], ones_row_f[:])

        def row_to_col(row_ap, dst_ap, scale=None):
            """[1, 128] bf16 SBUF row -> [128, 1] SBUF column via PE transpose.

            bf16 (single-pass weight load): a true-FP32 transpose here is a
            multi-pass FP32_HI weight load, which wedges the PE when
            interleaved with fp8 weight loads (HW hang, bisected on-device).
            """
            ps = ppt.tile([P, QC], BF16, tag="ppt", name="ptrc")
            nc.tensor.transpose(ps[:, 0:1], row_ap, ident_bf[0:1, 0:1])
            if scale is None:
                nc.vector.tensor_copy(dst_ap, ps[:, 0:1])
            else:
                nc.vector.tensor_scalar_mul(dst_ap, ps[:, 0:1], scale)

        def load_bias_row(nm):
            st = stage.tile([1, D], F32, tag="brow", name="brow")
            nc.sync.dma_start(st[:], b_aps[nm][None, :])
            return st

        def to_bf_row(row):
            st = stage.tile([1, D], BF16, tag="bfrow", name="bfrow")
            nc.vector.tensor_copy(st[:], row[0:1, :])
            return st

        def load_wbf(nm):
            """Weight (host-pre-cast bf16), natural [row, col] layout, one
            flat [128, ND*D] tile (row-tile-major), single batched DMA."""
            wt = wset.tile([P, ND * D], BF16, tag="wset", name=f"w{nm}")
            w3 = w_aps[nm].rearrange("(rt p) d -> p rt d", p=P)
            h = ND // 2
            for half in range(2):
                nc.sync.dma_start(
                    wt[:, D * h * half : D * h * (half + 1)].rearrange(
                        "p (rt d) -> p rt d", rt=h
                    ),
                    w3[:, h * half : h * (half + 1), :],
                )
            return wt

        # ---- one-time weight setup ----
        # A16: bf16 of HT_SC*SCALE*(Wq^T Wk), d-tile-major flat. The HT
        # matmul runs bf16 (same PE cycles as the fp8-DR Ah+Al error-
        # feedback pair it replaced, but kills both the A-quantization and
        # the query-side x8 error).
        A16 = ab_pool.tile([P, ND * D], BF16, tag="A16", name="A16")
        Bm = ab_pool.tile([P, ND * D], BF16, tag="Bm", name="Bm")
        # v16_col[:, dpt] = HT_SC*SCALE*(Wk^T bq)[dpt-block]: per-partition
        # bias added to HT at eviction (folds the w[k] exp bias into HT).
        v16_col = consts.tile([P, ND], F32, tag="v16_col")
        w_setup = {}

        def setup_part1(wq, wk):
            # A = Wq^T Wk ;  v16 = (Wk^T bq) * HT_SC * SCALE
            bq_row = load_bias_row("bq")
            for dt_ in range(ND):
                ps = pt_tile()
                for et in range(ND):
                    nc.tensor.matmul(
                        ps[:],
                        wq[:, D * et + P * dt_ : D * et + P * (dt_ + 1)],
                        wk[:, D * et : D * (et + 1)],
                        start=(et == 0),
                        stop=(et == ND - 1),
                    )
                sl = slice(D * dt_, D * (dt_ + 1))
                nc.vector.tensor_scalar_mul(A16[:, sl], ps[:], HT_SC * SCALE)
            bq_col = consts.tile([P, ND], BF16, tag="bq_col")
            bq_bf = to_bf_row(bq_row)
            for t in range(ND):
                row_to_col(bq_bf[0:1, P * t : P * (t + 1)], bq_col[:, t : t + 1])
            psv = pt_tile()
            for et in range(ND):
                nc.tensor.matmul(
                    psv[0:1, :],
                    bq_col[:, et : et + 1],
                    wk[:, D * et : D * (et + 1)],
                    start=(et == 0),
                    stop=(et == ND - 1),
                )
            v_row = stage.tile([1, D], BF16, tag="vrow", name="v_row")
            nc.vector.tensor_scalar_mul(v_row[:], psv[0:1, :], SCALE * HT_SC)
            for t in range(ND):
                row_to_col(v_row[0:1, P * t : P * (t + 1)], v16_col[:, t : t + 1])

        def setup_part2(wv, wo):
            # B = Wv^T Wo^T (bf16) ;  c = bv Wo^T + bo  (broadcast to 128 rows)
            woT = wset.tile([P, ND * D], BF16, tag="wset", name="WoT")
            for gt in range(ND):
                for ft in range(ND):
                    ps = ppt.tile([P, QC], BF16, tag="ppt", name="ptw")
                    nc.tensor.transpose(
                        ps[:, 0:P],
                        wo[:, D * gt + P * ft : D * gt + P * (ft + 1)],
                        ident_bf[:],
                    )
                    nc.vector.tensor_copy(
                        woT[:, D * ft + P * gt : D * ft + P * (gt + 1)], ps[:, 0:P]
                    )
            for dt_ in range(ND):
                ps = pt_tile()
                for ft in range(ND):
                    nc.tensor.matmul(
                        ps[:],
                        wv[:, D * ft + P * dt_ : D * ft + P * (dt_ + 1)],
                        woT[:, D * ft : D * (ft + 1)],
                        start=(ft == 0),
                        stop=(ft == ND - 1),
                    )
                nc.vector.tensor_copy(Bm[:, D * dt_ : D * (dt_ + 1)], ps[:])
            bv_row = load_bias_row("bv")
            bo_row = load_bias_row("bo")
            bv_col = stage.tile([P, ND], BF16, tag="bvcol", name="bv_col")
            bv_bf = to_bf_row(bv_row)
            for t in range(ND):
                row_to_col(bv_bf[0:1, P * t : P * (t + 1)], bv_col[:, t : t + 1])
            psc = pt_tile()
            for ft in range(ND):
                nc.tensor.matmul(
                    psc[0:1, :],
                    bv_col[:, ft : ft + 1],
                    woT[:, D * ft : D * (ft + 1)],
                    start=(ft == 0),
                    stop=(ft == ND - 1),
                )
            c_row = stage.tile([1, D], F32R, tag="crow", name="c_row")
            nc.vector.tensor_add(c_row[:], psc[0:1, :], bo_row[0:1, :])
            c_bf = consts.tile([1, D], BF16, tag="c_bf")
            nc.vector.tensor_copy(c_bf[:], c_row[0:1, :])
            w_setup["c_bf"] = c_bf
            psb = pt_tile()
            nc.tensor.matmul(psb[:], ones_row_r[:], c_row[:], start=True, stop=True)
            c_bc = consts.tile([P, D], F32, tag="c_bc")
            nc.vector.tensor_copy(c_bc[:], psb[:])
            w_setup["c_bc"] = c_bc

        # per-q-chunk epilogue. The PSUM-freeing evictions (ZT chunk -> SBUF
        # bf16, rowsum -> SBUF) are emitted immediately at chunk end; the
        # PE-side tail (1/rs transposes + y projection) is deferred into the
        # next chunk's kt-loop so the PE never drains between chunks.
        state = {"pending": None}

        def evict_chunk(b, qc, po, pr, final=False):
            rsrow = rs_pool.tile([1, QC], BF16, tag="rs", name="rsrow")
            nc.vector.tensor_copy(rsrow[:], pr[0:1, :])
            oc = oc_pool.tile([P, ND * QC], BF16, tag="oc", name="oc")
            # final chunk: ACT is idle after the last exp, DVE still has the
            # a8 cast + rsrow in its queue -- shift the balance to ACT
            act_dts = (1, 2, 3) if final else (1, 3)
            for dt_ in range(ND):
                sl = slice(QC * dt_, QC * (dt_ + 1))
                if dt_ in act_dts:
                    nc.scalar.activation(oc[:, sl], po[dt_][:], AFT.Copy)
                else:
                    nc.vector.tensor_copy(oc[:, sl], po[dt_][:])
            return (b, qc, oc, rsrow)

        def emit_epilogue(b, qc, oc, rsrow, final=False):
            rsT = rs_pool.tile([P, TPC], F32, tag="rsT", name="rsT")
            for j in range(TPC):
                row_to_col(rsrow[0:1, P * j : P * (j + 1)], rsT[:, j : j + 1])
            rsr = rs_pool.tile([P, TPC], F32, tag="rsr", name="rsr")
            nc.vector.reciprocal(rsr[:], rsT[:])
            # all 4 q-tiles accumulate into one flat SBUF tile; one batched
            # DMA (issued from the otherwise-idle gpsimd queue, off the busy
            # SP queue) writes the whole q-chunk back. The final chunk
            # instead streams 4 per-q-tile DMAs so writeback starts as soon
            # as the first tile's eviction lands.
            ysb = y_pool.tile([P, TPC * D], F32, tag="y", name="ysb")
            for j in range(TPC):
                act_evict = final and j % 2 == 1
                ps = pt_tile()
                for dt_ in range(ND):
                    nc.tensor.matmul(
                        ps[:],
                        oc[:, QC * dt_ + P * j : QC * dt_ + P * (j + 1)],
                        Bm[:, D * dt_ : D * (dt_ + 1)],
                        start=(dt_ == 0),
                        stop=(dt_ == ND - 1) and not act_evict,
                    )
                if act_evict:
                    # rank-1 rs (x) c into the PSUM (rs*c*rsr == c exactly up
                    # to one f32 reciprocal rounding), so ACT can evict with
                    # a per-partition scale -- in parallel with DVE's
                    # scalar_tensor_tensor on the even j's. Only worth it on
                    # the final chunk, where this chain is the kernel tail.
                    nc.tensor.matmul(
                        ps[:],
                        rsrow[0:1, P * j : P * (j + 1)],
                        w_setup["c_bf"][0:1, :],
                        start=False,
                        stop=True,
                    )
                    nc.scalar.activation(
                        ysb[:, D * j : D * (j + 1)], ps[:],
                        AFT.Copy, scale=rsr[:, j : j + 1],
                    )
                else:
                    nc.vector.scalar_tensor_tensor(
                        ysb[:, D * j : D * (j + 1)],
                        ps[:],
                        rsr[:, j : j + 1],
                        w_setup["c_bc"][:],
                        op0=ALU.mult,
                        op1=ALU.add,
                    )
                if final:
                    # alternate DMA queues so descriptor-issue time (~600ns
                    # each) doesn't serialize the drain tail
                    q = nc.gpsimd if j % 2 == 0 else nc.sync
                    q.dma_start(
                        y_ap[b, QC * qc + P * j : QC * qc + P * (j + 1), :],
                        ysb[:, D * j : D * (j + 1)],
                    )
            if not final:
                nc.gpsimd.dma_start(
                    y_ap[b, QC * qc : QC * (qc + 1), :].rearrange(
                        "(j p) d -> p j d", p=P
                    ),
                    ysb[:].rearrange("p (j d) -> p j d", j=TPC),
                )

        # ---- per batch residents ----
        # xT8: one flat fp8 [128, ND*S] tile per batch, d-tile-major: column
        # block dt*S + s holds x[s, dt*128+p]. One strided DVE copy evicts a
        # whole x-tile's 4 transposed blocks at once.
        # x8N: one flat fp8 [128, NS*D] tile per batch, s-tile-major: column
        # block i*D + d holds x[i*128+p, d]. DR-pair weights for the ZT
        # accumulation (pairs of adjacent s-tiles, stride D bytes).
        xTs = [
            xt_pool.tile([P, ND * S], F8, tag="xt", name=f"xT{b}")
            for b in range(BPC)
        ]
        x8Ns = [
            x8n_pool.tile([P, NS * D], F8, tag="x8n", name=f"x8N{b}")
            for b in range(BPC)
        ]
        def xt3(bb):
            return xTs[bb][:].rearrange("p (dt s) -> p dt s", dt=ND)

        # per-chunk transposed-bf16 x staging: written by x_process,
        # consumed exactly once by emit_ht (a full-batch resident would cost
        # 2x16KB/partition of SBUF; at most ~4 chunks are in flight)
        xtb_d = {}

        def xtb_tile(bb, sc):
            if (bb, sc) not in xtb_d:
                xtb_d[(bb, sc)] = xtb_pool.tile(
                    [P, ND * QC], BF16, tag="xtb", name="xTb"
                )
            return xtb_d[(bb, sc)]

        def x8n3(bb):
            return x8Ns[bb][:].rearrange("p (i d) -> p i d", i=NS)

        x_staged = {}  # (bb, sc) -> staged f32 tile (DMA issued, not processed)

        def x_dma(bb, sc):
            # issue one batched-descriptor DMA for a 512-wide s-chunk; the
            # PE/DVE-side processing is deferred (decouples DMA latency from
            # PE program order). x arrives host-pre-cast to bf16: half the
            # DMA bytes, and the PE transposes read the staged tile
            # DIRECTLY (no DVE staging cast in the transpose dependency
            # chain -- that cast was the batch-boundary stall mechanism).
            st = stage.tile([P, TPC * D], BF16, tag="xstage", name="xst", bufs=3)
            nc.sync.dma_start(
                st[:].rearrange("p (j d) -> p j d", j=TPC),
                x_ap[bb, QC * sc : QC * (sc + 1), :]
                .rearrange("(j p) d -> p j d", p=P),
            )
            x_staged[(bb, sc)] = st

        def x_process(bb, sc):
            # fp8-cast (x8N, DVE RNE) + bf16 PE transposes straight off
            # the staged bf16 tile. (fp8 PE transposes with fp8 PSUM out
            # pass CoreSim but are rejected by the walrus verifier; gpsimd
            # tensor_copy measures ~2us per [128,512] tile -- 5x slower
            # than DVE.)
            st = x_staged.pop((bb, sc))
            for j in range(TPC):
                i = TPC * sc + j
                nc.vector.tensor_copy(
                    x8n3(bb)[:, i, :], st[:, D * j : D * (j + 1)]
                )
                ps = ppt.tile([P, QC], BF16, tag="ppt", name="ptr")
                for dt_ in range(ND):
                    nc.tensor.transpose(
                        ps[:, P * dt_ : P * (dt_ + 1)],
                        st[:, D * j + P * dt_ : D * j + P * (dt_ + 1)],
                        ident_bf[:],
                    )
                # two evictions of the transpose PSUM: fp8 (DVE: RNE cast)
                # for the scores DR weights, bf16 (ACT: rounding-free byte
                # copy) for the HT matmul moving operand
                nc.vector.tensor_copy(
                    xt3(bb)[:, :, P * i : P * (i + 1)],
                    ps[:].rearrange("p (dt c) -> p dt c", dt=ND),
                )
                xtb = xtb_tile(bb, sc)
                nc.scalar.activation(
                    xtb[:].rearrange("p (dt c) -> p dt c", dt=ND)[
                        :, :, P * j : P * (j + 1)
                    ],
                    ps[:].rearrange("p (dt c) -> p dt c", dt=ND),
                    AFT.Copy,
                )

        # ---- JIT HT chunks ----
        HT_d = {}  # (b, hsc) -> fp8 flat [128, ND*QC] tile

        def emit_ht(b, hsc):
            # HT chunk: fp8 flat [128, ND*QC] holding 16*SCALE*(x A + v)
            # (v = the folded w[k] exp bias, added via the eviction ACT's
            # per-partition bias operand). bf16 matmuls over A16 x xTbf:
            # query-side x and A stay bf16-exact; only the HT output is fp8.
            # JIT, from inside the previous chunk's kt-loop, adjacent to the
            # bf16 epilogue island so the weight-dtype switch is shared.
            ht = ht_pool.tile([P, ND * QC], F8, tag="ht", name="HT")
            HT_d[(b, hsc)] = ht
            xtb = xtb_d.pop((b, hsc))[:].rearrange("p (dt c) -> p dt c", dt=ND)
            a163 = A16[:].rearrange("p (dt e) -> p dt e", dt=ND)
            for dpt in range(ND):
                ps = pt_tile()
                for dt_ in range(ND):
                    nc.tensor.matmul(
                        ps[:],
                        a163[:, dt_, P * dpt : P * (dpt + 1)],
                        xtb[:, dt_, :],
                        start=(dt_ == 0),
                        stop=(dt_ == ND - 1),
                    )
                # evictions alternate DVE/ACT so the consumer (the next
                # chunk's first scores pair, which reads all 4 dpt blocks)
                # waits ~half as long; scores jp=0 needs only blocks 0/1
                if dpt in (0, 2):
                    nc.vector.tensor_scalar_add(
                        ht[:, QC * dpt : QC * (dpt + 1)], ps[:],
                        v16_col[:, dpt : dpt + 1],
                    )
                else:
                    nc.scalar.activation(
                        ht[:, QC * dpt : QC * (dpt + 1)], ps[:],
                        AFT.Identity, scale=1.0,
                        bias=v16_col[:, dpt : dpt + 1],
                    )

        # ---- kt-pair machinery (shared across chunks for cross-chunk
        # software pipelining: a chunk's first scores pair is emitted from
        # the PREVIOUS chunk's tail) ----
        pss_d = {}   # (b, qc, kt) -> scores PSUM tile
        at8_d = {}   # (b, qc, pp) -> fp8 attention pair tile

        def scores(b, qc, kt):
            ps = pt_tile()
            ht3 = HT_d[(b, qc)][:].rearrange("p (dpt q) -> p dpt q", dpt=ND)
            xts = xt3(b)
            for jp in range(2):
                nc.tensor.matmul(
                    ps[:],
                    xts[:, 2 * jp : 2 * jp + 2, P * kt : P * (kt + 1)],
                    ht3[:, 2 * jp : 2 * jp + 2, :],
                    start=(jp == 0),
                    stop=(jp == 1),
                    perf_mode=DR,
                )
            pss_d[(b, qc, kt)] = ps

        def exp_pair(b, qc, pp):
            # fp8 attention weights a8[:, two*QC + q] = fp8(exp(pss/16)).
            # The one tile feeds the rowsum AND the ZT accumulation (same
            # quantized weights in numerator and denominator).
            a8 = at8_pool.tile([P, 2 * QC], F8, tag="at8", name="at8")
            if A8_DIRECT:
                for two in range(2):
                    ps = pss_d.pop((b, qc, 2 * pp + two))
                    nc.scalar.activation(
                        a8[:, QC * two : QC * (two + 1)], ps[:],
                        AFT.Exp, scale=1.0 / HT_SC,
                    )
            else:
                # f32 intermediate: ACT's native output path, so the only
                # rounding a8 sees is the DVE's single RNE f32->fp8 cast
                # (a bf16 intermediate double-rounds: +~0.1e-2 rel err)
                at = at_pool.tile([P, 2 * QC], F32, tag="at", name="at")
                split = (b, qc, pp) == (0, 0, 0)
                for two in range(2):
                    ps = pss_d.pop((b, qc, 2 * pp + two))
                    nc.scalar.activation(
                        at[:, QC * two : QC * (two + 1)], ps[:],
                        AFT.Exp, scale=1.0 / HT_SC,
                    )
                    if split:
                        nc.vector.tensor_copy(
                            a8[:, QC * two : QC * (two + 1)],
                            at[:, QC * two : QC * (two + 1)],
                        )
                if not split:
                    nc.vector.tensor_copy(a8[:], at[:])
            at8_d[(b, qc, pp)] = a8

        def po_dr(b, qc, pp, po):
            # ZT accumulation for pair pp: fp8 DR, weights = x8N s-tile
            # pairs, moving = the pair's a8 tile.
            x8n = x8n3(b)
            a2 = at8_d.pop((b, qc, pp))[:].rearrange("p (two q) -> p two q", two=2)
            for dt_ in range(ND):
                nc.tensor.matmul(
                    po[dt_][:],
                    x8n[:, 2 * pp : 2 * pp + 2, P * dt_ : P * (dt_ + 1)],
                    a2,
                    start=(pp == 0),
                    stop=(pp == NP - 1),
                    perf_mode=DR,
                )

        def pr_dr(b, qc, pp, pr):
            # rowsum for pair pp: one fp8 DR matmul, 256-col all-ones
            # weights (all output rows identical; 1-col weight loads stall
            # the PE weight-load pipeline ~110ns, so full-width it is).
            nc.tensor.matmul(
                pr[:],
                ones8[:].rearrange("p (two m) -> p two m", two=2),
                at8_d[(b, qc, pp)][:].rearrange("p (two q) -> p two q", two=2),
                start=(pp == 0),
                stop=(pp == NP - 1),
                perf_mode=DR,
            )

        # ---- main loop over (batch, chunk) jobs ----
        jobs = [(b, qc) for b in range(BPC) for qc in range(NQC)]
        for ji, (b, qc) in enumerate(jobs):
            nxt = jobs[ji + 1] if ji + 1 < len(jobs) else None
            if b == 0 and qc == 0:
                # startup. Sync-queue DMA order: Wq, Wk, bq (A = Wq^T Wk and
                # v = Wk^T bq head the longest dependency chain: A/v ->
                # HT(0) -> scores(0)), then the 4 x chunks, then Wv/Wo
                # (first needed at p==4). PE order: A/v setup (waits only on
                # the small leading DMAs), then chunk 0's transposes, HT(0),
                # first scores pair. Chunks 1-3 process inside the kt loop
                # (keys of chunk c are first touched at pair 2c-1).
                wq = load_wbf("Wq")
                wk = load_wbf("Wk")
                setup_part1(wq, wk)
                for sc_ in range(NQC):
                    x_dma(0, sc_)
                w_setup["wv"] = load_wbf("Wv")
                w_setup["wo"] = load_wbf("Wo")
                x_process(0, 0)
                emit_ht(0, 0)
                scores(b, qc, 0)
                scores(b, qc, 1)
            po = [
                ppo.tile([P, QC], F32, tag="ppo", name="po") for _ in range(ND)
            ]
            pr = ppr.tile([P, QC], F32, tag="ppr", name="pr")
            for p in range(NP):
                k0 = 2 * p
                # ACT: exp of the current pair (overlaps the PE blocks)
                exp_pair(b, qc, p)
                # the PREVIOUS chunk's deferred last-pair rowsum/ZT + PSUM
                # evictions run here, ~4us after that pair's exp was issued
                # (doing them in the previous chunk's own tail stalled the
                # PE ~0.6us per chunk on the exp->cast chain)
                if p == 0 and state.get("tail") is not None:
                    tb, tqc, tpo, tpr = state.pop("tail")
                    pr_dr(tb, tqc, NP - 1, tpr)
                    po_dr(tb, tqc, NP - 1, tpo)
                    state["pending"] = evict_chunk(tb, tqc, tpo, tpr)
                # fp8-DR block: next pair's scores + prev pair's rowsum
                # + prev pair's ZT
                if k0 + 2 < NS:
                    scores(b, qc, k0 + 2)
                    scores(b, qc, k0 + 3)
                if p >= 1:
                    pr_dr(b, qc, p - 1, pr)
                    po_dr(b, qc, p - 1, po)
                # bf16 island at p==1: deferred epilogue (y matmuls + rs
                # transposes) and the JIT bf16 HT block for the next chunk
                # share one weight-dtype switch pair
                if p == 1 and state["pending"] is not None:
                    emit_epilogue(*state["pending"])
                    state["pending"] = None
                if p == 1 and qc + 1 < NQC:
                    emit_ht(b, qc + 1)
                # b0 ramp: chunks 1-3 stream in under the qc0 kt-loop
                # (chunk c's keys are first touched at pair 2c-1)
                if b == 0 and qc == 0 and p in (0, 1, 3):
                    x_process(0, {0: 1, 1: 2, 3: 3}[p])
                # B / c are first needed by qc0's epilogue (flushed at
                # qc1 p==1): compute them inside qc0's dense kt-loop
                if b == 0 and qc == 0 and p == 4:
                    setup_part2(w_setup.pop("wv"), w_setup.pop("wo"))
                # prefetch ALL of the next batch's x chunks into the tail
                # of this batch's last attention chunk: DMAs issue up front
                # (p 0/1), processing staggers at p 2/3/5/6, and the next
                # batch's HT(0) follows its chunk-0 processing.
                if qc == NQC - 1 and b + 1 < BPC:
                    if p in (0, 1):
                        x_dma(b + 1, 2 * p)
                        x_dma(b + 1, 2 * p + 1)
                    if p in (2, 3, 5, 6):
                        x_process(b + 1, {2: 0, 3: 1, 5: 2, 6: 3}[p])
                    if p == 3:
                        # rides the p==3 x_process bf16 island; needs only
                        # the chunk-0 prefetch (processed at p==2)
                        emit_ht(b + 1, 0)
            # tail: emit the NEXT chunk's first scores pair; this chunk's
            # last-pair rowsum/ZT + evictions defer into the next chunk's
            # p==0 block (see above). The final chunk does them inline.
            if nxt is not None:
                scores(nxt[0], nxt[1], 0)
                scores(nxt[0], nxt[1], 1)
                state["tail"] = (b, qc, po, pr)
            else:
                pr_dr(b, qc, NP - 1, pr)
                po_dr(b, qc, NP - 1, po)
                state["pending"] = evict_chunk(b, qc, po, pr, final=True)

        if state["pending"] is not None:
            emit_epilogue(*state["pending"], final=True)
            state["pending"] = None


def build_program():
    nc = bacc.Bacc("TRN2", target_bir_lowering=False, debug=False)
    x_ap = nc.dram_tensor("x", [BPC, S, D], BF16, kind="ExternalInput").ap()
    w_aps = {
        nm: nc.dram_tensor(nm, [D, D], BF16, kind="ExternalInput").ap()
        for nm in ("Wq", "Wk", "Wv", "Wo")
    }
    b_aps = {
        nm: nc.dram_tensor(nm, [D], F32, kind="ExternalInput").ap()
        for nm in ("bq", "bk", "bv", "bo")
    }
    y_ap = nc.dram_tensor("y", [BPC, S, D], F32, kind="ExternalOutput").ap()
    with tile.TileContext(nc) as tc:
        _emit(tc, x_ap, w_aps, b_aps, y_ap)
    nc.compile()
    return nc


_program_cache = {}


def _get_program(fast_mm=True):
    # fast_mm retained for test.py compatibility; single fp8/bf16 program.
    if "p" not in _program_cache:
        _program_cache["p"] = build_program()
    return _program_cache["p"]


def _make_in_maps(inputs):
    arrs = {
        k: np.ascontiguousarray(np.asarray(v, dtype=np.float32))
        for k, v in inputs.items()
    }
    x_bf = np.ascontiguousarray(arrs["x"].astype(ml_dtypes.bfloat16))
    in_maps = []
    for core in range(N_CORES):
        m = {"x": x_bf[BPC * core : BPC * (core + 1)]}
        for nm in ("Wq", "Wk", "Wv", "Wo"):
            # host-side bf16 pre-cast (same RNE rounding the on-device DVE
            # cast applied): halves the startup weight DMA bytes
            m[nm] = np.ascontiguousarray(arrs[nm].astype(ml_dtypes.bfloat16))
        for nm in ("bq", "bk", "bv", "bo"):
            m[nm] = arrs[nm]
        in_maps.append(m)
    return in_maps


def run(inputs, fast_mm=True, trace=False):
    """Returns (y_full, BassKernelResults)."""
    nc = _get_program(fast_mm)
    in_maps = _make_in_maps(inputs)
    last_err = None
    for attempt in range(3):
        try:
            res = run_bass_kernel_spmd(nc, in_maps, list(range(N_CORES)), trace=trace)
            break
        except Exception as e:  # transient NRT device errors: retry
            last_err = e
            import time

            time.sleep(2.0 * (attempt + 1))
    else:
        raise last_err
    y = np.concatenate([r["y"] for r in res.results], axis=0)
    return np.ascontiguousarray(y.astype(np.float32)), res


def kernel(**inputs):
    y, _ = run(inputs, trace=False)
    return y
